# revision 32
# baseline (speedup 1.0000x reference)
"""Causal single-head attention on 8 TRN2 NeuronCores (Bass/Tile).

Problem: inputs [B=4, S=2048, D=1024] f32; WK/WQ/WV [1024, 1024] f32.
  K/Q/V = X @ W*; scores = Q K^T / sqrt(D); causal mask; softmax; out = P V.

Sharding: 8 cores = (batch b, half h); core (b, h) owns q-tile
2j + (h if j even else 1-h) in slot j (h=0 -> tiles 0,3,4,7,...; h=1 ->
1,2,5,6,...), which balances causal work; slot j covers k-tiles 0..2j+1 on
every core (identical SPMD program), with the exact causal boundary
enforced by a per-core, slot-parity-selected additive mask accumulated
into the scores PSUM via an identity-stationary matmul.

Communication-minimal projection layout (one collective total):
  - K: each core computes only its key half (a partition-parity-dependent
    dynamic-offset window of the full X^T input) and the pair exchanges the
    halves with a single intra-pair AllGather, which hides behind the V/Q
    projections.
  - V: recomputed fully on every core from the full X^T input (the host
    supplies X^T to both cores of a pair, so this replaces what would be a
    second, ~60us serialized AllGather with overlapped matmul work).
  - Q: computed locally for the core's own q-rows.

Matmuls run in bf16 with fp32 PSUM accumulation; softmax runs without
max-subtraction, row sums come free from the Exp activation's accum_out,
and normalization is folded into the output PSUM eviction as a per-
partition scale. The output is stored bf16 and upcast on the host.

Scheduling: DMA traffic is split by class across the DGE queues (inputs on
SP, K-half evictions and output on ACT, the collective on Pool) so no queue
blocks another's critical path. The attention loop is a one-slot software
pipeline -- slot ordering is largest-first so the pipeline drain is the
smallest slot, the previous slot's P^T transposes are interleaved into the
next slot's score matmuls to hide PSUM->SBUF copy latency, and exp (ACT)
runs under the previous slot's P^T V matmuls.

Measured (K-repeat slope on 8 axon-tunneled NC_v3 cores): ~131 us/iter in
short bursts (power-unthrottled), ~233 us/iter sustained; the previous
baseline measured ~231 us sustained and ~241 us in bursts (it was
collective-latency-bound; this kernel is tensor-engine-bound).
"""

from concourse.bass_utils import run_bass_kernel_spmd


from contextlib import ExitStack
from math import ceil

import ml_dtypes
import numpy as np

import concourse.mybir as mybir
import concourse.tile as tile
from concourse import bacc
from concourse.bass import ds

BF = mybir.dt.bfloat16
F32 = mybir.dt.float32
NEG = -1e9


def build_nc_pair(
    KD=8, ED=8, n_slots=8, S=2048, chunk=512, repeat=1, groups=None,
    no_collective=False, xsplit=1, skip_attention=False, extra_loads=0,
):
    NQ = n_slots * 128
    DO = ED * 128
    n_ktiles = S // 128
    SH = S // 2  # own key half
    n_ktiles_h = n_ktiles // 2
    scale = 1.0 / np.sqrt(np.float32(KD * 128))

    nc = bacc.Bacc(None, target_bir_lowering=False, debug=False)

    xtf_d = nc.dram_tensor("xtf", [KD, 128, S], BF, kind="ExternalInput")
    xtq_d = nc.dram_tensor("xtq", [KD, 128, NQ], BF, kind="ExternalInput")
    wk_d = nc.dram_tensor("wk", [KD, 128, DO], BF, kind="ExternalInput")
    wq_d = nc.dram_tensor("wq", [KD, 128, DO], BF, kind="ExternalInput")
    wv_d = nc.dram_tensor("wv", [KD, 128, DO], BF, kind="ExternalInput")
    mask_d = nc.dram_tensor("mask", [2, 128, 256], BF, kind="ExternalInput")
    out_d = nc.dram_tensor("out", [n_slots, 128, DO], BF, kind="ExternalOutput")

    ident_d = nc.inline_tensor(np.eye(128).astype(ml_dtypes.bfloat16), "ident")
    if groups is None:
        groups = [[0, 1], [2, 3], [4, 5], [6, 7]]

    with tile.TileContext(nc) as tc, ExitStack() as ctx:
      persist = ctx.enter_context(tc.tile_pool(name="persist", bufs=1))
      pp = ctx.enter_context(tc.tile_pool(name="pp", bufs=6, space="PSUM"))
      ptp = ctx.enter_context(tc.tile_pool(name="ptp", bufs=2, space="PSUM"))
      att = ctx.enter_context(tc.tile_pool(name="att", bufs=2))
      pts = ctx.enter_context(tc.tile_pool(name="pts", bufs=36))
      dram = ctx.enter_context(tc.tile_pool(name="dram", bufs=1, space="DRAM"))
      stage = ctx.enter_context(tc.tile_pool(name="stage", bufs=6))
      for _rep in range(repeat):
        XTF = persist.tile([128, KD, S], BF, tag="XTF")
        XTQ = persist.tile([128, KD, NQ], BF, tag="XTQ")
        WK = persist.tile([128, KD, DO], BF, tag="WK")
        WQ = persist.tile([128, KD, DO], BF, tag="WQ")
        WV = persist.tile([128, KD, DO], BF, tag="WV")
        KT = persist.tile([128, ED, S], BF, tag="KT")
        V = persist.tile([128, n_ktiles, DO], BF, tag="V")
        QT = persist.tile([128, ED, NQ], BF, tag="QT")
        maskt = persist.tile([128, 2, 256], BF, tag="maskt")
        ident = persist.tile([128, 128], BF, tag="ident")

        # exchange buffers, split in two pieces each so collectives start as
        # soon as half the projection has been evicted
        if xsplit == 2:
            kt_sh_own = [
                dram.tile(
                    [ED, 128, SH // 2], BF, tag=f"kt_sh_own{c}", name=f"kt_sh_own{c}"
                )
                for c in range(2)
            ]
            kt_sh_all = [
                dram.tile(
                    [2, ED, 128, SH // 2], BF, tag=f"kt_sh_all{c}",
                    name=f"kt_sh_all{c}",
                )
                for c in range(2)
            ]

        else:
            # merged buffers: one AllGather per tensor, issued after the full
            # projection ([c/g]-indexed slices of a single DRAM tile)
            kt_sh_own_m = dram.tile(
                [2, ED, 128, SH // 2], BF, tag="kt_sh_own", name="kt_sh_own_m"
            )
            kt_sh_all_m = dram.tile(
                [2, 2, ED, 128, SH // 2], BF, tag="kt_sh_all", name="kt_sh_all_m"
            )
            kt_sh_own = [kt_sh_own_m[c] for c in range(2)]
            # kt_sh_all_m layout: [h2, c, m, 128, cols]
            kt_sh_all = [kt_sh_all_m[:, c] for c in range(2)]


        # -- input loads: SP queue only --
        nc.sync.dma_start(out=ident, in_=ident_d[:])
        for p in range(2):
            nc.sync.dma_start(out=maskt[:, p, :], in_=mask_d[p])
        # WK on SP, X^T split across SP/ACT so both queues stream the
        # K-projection's inputs concurrently (halves the startup gating)
        for kd in range(KD):
            nc.sync.dma_start(out=WK[:, kd, :], in_=wk_d[kd])
            if kd % 2 == 0:
                nc.sync.dma_start(out=XTF[:, kd, :], in_=xtf_d[kd])
            else:
                nc.scalar.dma_start(out=XTF[:, kd, :], in_=xtf_d[kd])
        for kd in range(KD):
            nc.sync.dma_start(out=WV[:, kd, :], in_=wv_d[kd])
        for kd in range(KD):
            nc.sync.dma_start(out=WQ[:, kd, :], in_=wq_d[kd])
            nc.sync.dma_start(out=XTQ[:, kd, :], in_=xtq_d[kd])
        for _x in range(extra_loads):
            for kd in range(KD):
                nc.sync.dma_start(out=XTF[:, kd, :], in_=xtf_d[kd])

        def proj(dst, lhs_buf, rhs_buf, n_m, n_free, lhs_of_m, rhs_of_c, sink=None):
            nch = ceil(n_free / chunk)
            for m in range(n_m):
                psums = [
                    pp.tile([128, chunk], F32, tag="pp", name="pp")
                    for _ in range(nch)
                ]
                for kd in range(KD):
                    lhsT = lhs_of_m(lhs_buf, kd, m)
                    for c in range(nch):
                        w = min(chunk, n_free - c * chunk)
                        nc.tensor.matmul(
                            psums[c][:, :w],
                            lhsT,
                            rhs_of_c(rhs_buf, kd, c, w),
                            start=(kd == 0),
                            stop=(kd == KD - 1),
                        )
                for c in range(nch):
                    w = min(chunk, n_free - c * chunk)
                    if sink is None:
                        nc.vector.tensor_copy(
                            dst[:, m, c * chunk : c * chunk + w], psums[c][:, :w]
                        )
                    else:
                        st = stage.tile([128, chunk], BF, tag="stage", name="stage")
                        nc.vector.tensor_copy(st[:, :w], psums[c][:, :w])
                        sink(m, c, w, st)

        wslice = lambda buf, kd, m: buf[:, kd, m * 128 : (m + 1) * 128]
        xslice = lambda buf, kd, c, w: buf[:, kd, c * chunk : c * chunk + w]

        def exchange(own, all_, ins_ap, outs_ap):
            if no_collective:
                for h2 in range(2):
                    nc.gpsimd.dma_start(out=all_[h2], in_=own[:])
            else:
                nc.gpsimd.collective_compute(
                    "AllGather",
                    mybir.AluOpType.bypass,
                    replica_groups=groups,
                    ins=[ins_ap],
                    outs=[outs_ap],
                )

        # KT_own = WK.T @ X_half: the own key half is a dynamic
        # (partition-parity dependent) column window of the full X^T
        pid = nc.tensor.partition_id()
        hofs = (pid % 2) * SH
        for c in range(2):
            for m in range(ED):
                ps = pp.tile([128, chunk], F32, tag="pp", name="pp")
                for kd in range(KD):
                    nc.tensor.matmul(
                        ps,
                        WK[:, kd, m * 128 : (m + 1) * 128],
                        XTF[:, kd, ds(hofs + c * chunk, chunk)],
                        start=(kd == 0),
                        stop=(kd == KD - 1),
                    )
                st = stage.tile([128, chunk], BF, tag="stage", name="stage")
                nc.vector.tensor_copy(st, ps)
                nc.scalar.dma_start(out=kt_sh_own[c][m], in_=st)
            if xsplit == 2:
                exchange(
                    kt_sh_own[c], kt_sh_all[c], kt_sh_own[c][:], kt_sh_all[c][:]
                )
        if xsplit == 1:
            if no_collective:
                for h2 in range(2):
                    nc.gpsimd.dma_start(out=kt_sh_all_m[h2], in_=kt_sh_own_m[:])
            else:
                nc.gpsimd.collective_compute(
                    "AllGather",
                    mybir.AluOpType.bypass,
                    replica_groups=groups,
                    ins=[kt_sh_own_m[:]],
                    outs=[kt_sh_all_m[:]],
                )
        for c in range(2):
            for h2 in range(2):
                for m in range(ED):
                    nc.sync.dma_start(
                        out=KT[
                            :, m, h2 * SH + c * chunk : h2 * SH + (c + 1) * chunk
                        ],
                        in_=kt_sh_all[c][h2, m],
                    )

        # V = X @ WV recomputed fully on every core from the full X^T
        # input (replaces the second AllGather); PSUM evicted straight into
        # the global-ordered V tile.
        ndc = ceil(DO / chunk)
        for m in range(n_ktiles):
            psums = [
                pp.tile([128, chunk], F32, tag="pp", name="pp")
                for _ in range(ndc)
            ]
            for kd in range(KD):
                lhsT = XTF[:, kd, m * 128 : (m + 1) * 128]
                for cc in range(ndc):
                    nc.tensor.matmul(
                        psums[cc],
                        lhsT,
                        WV[:, kd, cc * chunk : (cc + 1) * chunk],
                        start=(kd == 0),
                        stop=(kd == KD - 1),
                    )
            for cc in range(ndc):
                nc.vector.tensor_copy(
                    V[:, m, cc * chunk : (cc + 1) * chunk], psums[cc]
                )

        # QT (psum -> SBUF directly, no DMA)
        proj(QT, WQ, XTQ, ED, NQ, wslice, xslice)

        if skip_attention:
            for j in range(n_slots):
                ob = att.tile([128, DO], BF, tag="out")
                nc.vector.tensor_copy(ob, QT[:, j, :])
                nc.scalar.dma_start(out=out_d[j], in_=ob)
            continue

        # -- attention: 1-slot software pipeline --
        # PE order per step: [scores_j interleaved with slot j-1 transposes],
        # then PV_{j-1}; exp_j (ACT) runs during PV_{j-1}. V is local, so PV
        # never waits on communication; only the KT AllGather gates scores.
        ndc = ceil(DO / chunk)

        def emit_transpose(P, kt, pt_tiles):
            tp = ptp.tile([128, 128], BF, tag="ptp", name="ptp")
            nc.tensor.transpose(tp, P[:, kt * 128 : (kt + 1) * 128], ident)
            pt_sb = pts.tile([128, 128], BF, tag="pt", name="pt")
            nc.vector.tensor_copy(pt_sb, tp)
            pt_tiles[kt] = pt_sb

        def emit_tail(j, pt_tiles, recip):
            nkt = 2 * (j + 1)
            opsums = [
                pp.tile([128, chunk], F32, tag="pp", name="pp") for _ in range(ndc)
            ]
            for kt in range(nkt):
                for c in range(ndc):
                    nc.tensor.matmul(
                        opsums[c],
                        pt_tiles[kt],
                        V[:, kt, c * chunk : (c + 1) * chunk],
                        start=(kt == 0),
                        stop=(kt == nkt - 1),
                    )
            out_sb = att.tile([128, DO], BF, tag="out")
            for c in range(ndc):
                nc.scalar.mul(
                    out_sb[:, c * chunk : (c + 1) * chunk],
                    opsums[c],
                    mul=recip,
                )
            nc.scalar.dma_start(out=out_d[j], in_=out_sb)

        prev = None  # (j, P, pt_tiles, recip) of the previous slot

        slot_order = list(range(n_slots - 1, -1, -1))
        for j in slot_order:
            nk = 256 * (j + 1)
            chunks = [
                (c * chunk, min(chunk, nk - c * chunk))
                for c in range(ceil(nk / chunk))
            ]
            nch = len(chunks)

            P = att.tile([128, S], BF, tag="P")
            sums = att.tile([128, 4], F32, tag="sums")

            if prev is not None:
                pj, pP, ppt, prec = prev
                pending = list(range(2 * (pj + 1)))
            else:
                pending = []

            spsums = [
                pp.tile([128, chunk], F32, tag="pp", name="pp") for _ in range(nch)
            ]
            for e in range(ED):
                lhsT = QT[:, e, j * 128 : (j + 1) * 128]
                for ci, (off, w) in enumerate(chunks):
                    nc.tensor.matmul(
                        spsums[ci][:, :w],
                        lhsT,
                        KT[:, e, off : off + w],
                        start=(e == 0),
                        stop=(e == ED - 1),
                    )
                    if pending:
                        emit_transpose(pP, pending.pop(0), ppt)
            loc = (nk - 256) - chunks[-1][0]
            nc.tensor.matmul(
                spsums[-1][:, loc : loc + 256],
                ident,
                maskt[:, j % 2, :],
                start=False,
                stop=False,
                skip_group_check=True,
            )
            while pending:
                emit_transpose(pP, pending.pop(0), ppt)

            for ci, (off, w) in enumerate(chunks):
                nc.scalar.activation(
                    P[:, off : off + w],
                    spsums[ci][:, :w],
                    mybir.ActivationFunctionType.Exp,
                    scale=float(scale),
                    accum_out=sums[:, ci : ci + 1],
                )

            total = att.tile([128, 1], F32, tag="total")
            nc.vector.reduce_sum(total, sums[:, :nch], axis=mybir.AxisListType.X)
            recip = att.tile([128, 1], F32, tag="recip")
            nc.vector.reciprocal(recip, total)

            if prev is not None:
                emit_tail(prev[0], prev[2], prev[3])

            pt_tiles = [None] * (2 * (j + 1))
            prev = (j, P, pt_tiles, recip)

        # drain: last (smallest) slot -- interleave transposes into PV
        pj, pP, ppt, prec = prev
        nkt = 2 * (pj + 1)
        for kt in range(min(2, nkt)):
            emit_transpose(pP, kt, ppt)
        opsums = [
            pp.tile([128, chunk], F32, tag="pp", name="pp") for _ in range(ndc)
        ]
        for kt in range(nkt):
            for c in range(ndc):
                nc.tensor.matmul(
                    opsums[c],
                    ppt[kt],
                    V[:, kt, c * chunk : (c + 1) * chunk],
                    start=(kt == 0),
                    stop=(kt == nkt - 1),
                )
            if kt + 2 < nkt:
                emit_transpose(pP, kt + 2, ppt)
        out_sb = att.tile([128, DO], BF, tag="out")
        for c in range(ndc):
            nc.scalar.mul(
                out_sb[:, c * chunk : (c + 1) * chunk], opsums[c], mul=prec
            )
        nc.scalar.dma_start(out=out_d[pj], in_=out_sb)

    nc.compile()
    return nc


def host_inputs_for_core_pair(X, WKn, WQn, WVn, core, n_slots):
    b, h = core // 2, core % 2
    S = X.shape[1]
    D = X.shape[2]
    KD = D // 128
    NQ = n_slots * 128
    SH = S // 2
    bf = ml_dtypes.bfloat16

    qtiles = [2 * j + (h if j % 2 == 0 else 1 - h) for j in range(n_slots)]
    qrows = np.concatenate([np.arange(t * 128, (t + 1) * 128) for t in qtiles])

    xtf = np.ascontiguousarray(X[b].T.astype(bf)).reshape(KD, 128, S)
    xtq = np.ascontiguousarray(X[b][qrows].T.astype(bf)).reshape(KD, 128, NQ)

    def wtile(W):
        return np.ascontiguousarray(W.astype(bf)).reshape(KD, 128, -1)

    r = np.arange(128)
    tri = np.where(r[None, :] <= r[:, None], 0.0, NEG).astype(np.float32)
    mA = np.zeros((128, 256), dtype=np.float32)
    mA[:, 128:] = tri  # diagonal tile is the last covered tile
    mB = np.zeros((128, 256), dtype=np.float32)
    mB[:, :128] = tri  # diagonal tile is second-from-last; last fully masked
    mB[:, 128:] = NEG
    # slot parity p uses mask (h==0: [B, A][p], h==1: [A, B][p])
    mask = np.stack([mB, mA] if h == 0 else [mA, mB])
    return {
        "xtf": xtf,
        "xtq": xtq,
        "wk": wtile(WKn),
        "wq": wtile(WQn),
        "wv": wtile(WVn),
        "mask": mask.astype(bf),
    }


B, S, D_IN, D_OUT = 4, 2048, 1024, 1024
N_SLOTS = 8

_NC_CACHE = []


def _get_nc():
    if not _NC_CACHE:
        _NC_CACHE.append(build_nc_pair())
    return _NC_CACHE[0]


def _host_inputs_for_core(X, WKn, WQn, WVn, core):
    return host_inputs_for_core_pair(X, WKn, WQn, WVn, core, N_SLOTS)


def build_nc(repeat=1):
    return build_nc_pair(repeat=repeat)


def kernel(inputs, WK, WQ, WV):
    X = np.asarray(inputs, dtype=np.float32)
    WKn = np.asarray(WK, dtype=np.float32)
    WQn = np.asarray(WQ, dtype=np.float32)
    WVn = np.asarray(WV, dtype=np.float32)

    nc = _get_nc()
    in_maps = [_host_inputs_for_core(X, WKn, WQn, WVn, c) for c in range(8)]
    res = run_bass_kernel_spmd(nc, in_maps, core_ids=list(range(8)))

    out = np.zeros((B, S, D_OUT), dtype=np.float32)
    for core in range(8):
        b, h = core // 2, core % 2
        o = np.asarray(res.results[core]["out"], dtype=np.float32)
        for j in range(N_SLOTS):
            t = 2 * j + (h if j % 2 == 0 else 1 - h)
            out[b, t * 128 : (t + 1) * 128, :] = o[j]
    return out
